# revision 1
# baseline (speedup 1.0000x reference)
"""Conv2D-Shakeout Trainium2 kernel.

Reference computation (NCHW, OIHW, stride 1, pad 1):
    mask  = (0.5 > r), imask = (r > 0.5)
    w' = 0.7*sign(w*imask) + 0.5*(w + 0.35*sign(w*mask))
       = 0.5*w + sign(w) * (0.7*imask + 0.175*mask)
    y = conv2d(x, w') + bias

Strategy (8 NeuronCores, data-parallel over batch, 4 images/core):
  - Host pads x width 56 -> 58 (zero cols) so every conv tap is a pure
    offset window in the [56, 58]-strided SBUF layout.
  - Shakeout transform + weight transpose (PE transpose) on device; the
    36 lhsT tiles ([ci 128, co 128] per (co-chunk, ci-chunk, tap)) are
    stored as float32r: full-rate (1 cycle/row) fp32 matmuls, ~1.5e-4
    rel err at K=2304 (hardware-measured).
  - Conv = 9 shifted matmuls x 2 ci-chunks accumulated in one PSUM bank
    per output tile [co 128, 8 rows x 56 cols]; row-clipped taps write
    contiguous PSUM subranges (full-coverage dy=0 taps issued first).
  - VectorE fuses bias-add with PSUM->SBUF drain; DMA out per tile.
    Emission order is tuned so the DMA queue (one shared-bandwidth
    resource) serves the head-critical bytes first and conv-group
    drains always outrank later prologue work on VectorE.
"""
from contextlib import ExitStack

import numpy as np

import concourse.bass as bass
import concourse.mybir as mybir
import concourse.tile as tile
from concourse import bacc
from concourse.bass_utils import run_bass_kernel_spmd
from concourse.masks import make_identity

F32 = mybir.dt.float32
F32R = mybir.dt.float32r
ACT = mybir.ActivationFunctionType
ALU = mybir.AluOpType

N_CORES = 8
B_SHARD = 4          # images per core
CIN = COUT = 256
H = W = 56
WP = 58              # host-padded width
KH = KW = 3
NCH = 2              # 128-partition channel chunks
BLK = 8              # output rows per PSUM tile
NBLK = H // BLK      # 7
TILE_N = BLK * W     # 448

TAU, C = 0.5, 0.7
ITAU = 1.0 - TAU

# dy=0 (full row coverage) taps first: the start=True matmul must cover
# every byte of its PSUM accumulation region.
TAP_ORDER = [(0, 0), (0, 1), (0, 2), (-1, 0), (-1, 1), (-1, 2), (1, 0), (1, 1), (1, 2)]


def _build(repeat=1):
    nc = bacc.Bacc("TRN2", target_bir_lowering=False, debug=False,
                   enable_asserts=False, num_devices=N_CORES)
    xp_d = nc.dram_tensor("xp", [B_SHARD, CIN, H, WP], F32, kind="ExternalInput").ap()
    w_d = nc.dram_tensor("w", [COUT, CIN, KH, KW], F32, kind="ExternalInput").ap()
    b_d = nc.dram_tensor("b", [COUT], F32, kind="ExternalInput").ap()
    r_d = nc.dram_tensor("r", [COUT, CIN, KH, KW], F32, kind="ExternalInput").ap()
    y_d = nc.dram_tensor("y", [B_SHARD, COUT, H, W], F32, kind="ExternalOutput").ap()

    with tile.TileContext(nc) as tc, ExitStack() as ctx:
        const = ctx.enter_context(tc.tile_pool(name="const", bufs=1))
        wtp = ctx.enter_context(tc.tile_pool(name="wtp", bufs=1))
        lhs = ctx.enter_context(tc.tile_pool(name="lhs", bufs=1))
        xpool = ctx.enter_context(tc.tile_pool(name="xpool", bufs=1))
        opool = ctx.enter_context(tc.tile_pool(name="opool", bufs=6))
        tps = ctx.enter_context(tc.tile_pool(name="tps", bufs=2, space="PSUM"))
        cps = ctx.enter_context(tc.tile_pool(name="cps", bufs=6, space="PSUM"))

        ident = const.tile([128, 128], F32)
        make_identity(nc, ident[:])
        bias_sb = const.tile([128, NCH], F32)
        nc.sync.dma_start(bias_sb[:], b_d.rearrange("(c p) -> p c", p=128))

        def body(iv=None, unroll=None):
            lhsT = lhs.tile([128, NCH * NCH * 9, 128], F32R, tag="lhsT")
            wq = w_d.rearrange("(c p) i kh kw -> p c (i kh kw)", p=128)
            rq = r_d.rearrange("(c p) i kh kw -> p c (i kh kw)", p=128)
            QF = 128 * 9  # transform quarter: one (co-chunk, ci-chunk) pair
            xv = {}

            def load_x(img, pieces=1):
                # pieces > 1: row-chunks interleaved across the two ci-chunks
                # so early conv blocks (which need BOTH chunks) start sooner
                for k in range(NCH):
                    xt = xpool.tile([128, H, WP], F32R, tag=f"x_{img}_{k}")
                    xv[img, k] = xt
                rows = H // pieces
                for q in range(pieces):
                    rs = slice(q * rows, (q + 1) * rows)
                    for k in range(NCH):
                        nc.sync.dma_start(
                            xv[img, k][:, rs],
                            xp_d[img, k * 128:(k + 1) * 128, rs].bitcast(F32R))

            def transform_chain(c, k, pieces=1):
                # shakeout transform: wp = 0.5*w + sign(w)*s
                # pieces=2 pipelines DMA + DVE chain halves (shorter head)
                sl = slice(k * QF, (k + 1) * QF)
                w_sb = wtp.tile([128, QF], F32, tag=f"w_sb{k}")
                r_sb = wtp.tile([128, QF], F32, tag=f"r_sb{k}")
                wp = wtp.tile([128, QF], F32, tag=f"wp{k}")
                PF = QF // pieces
                for q in range(pieces):
                    # r before w: the DVE chain head (s1/s2) needs only r;
                    # Sign(w) runs on ACT in parallel and is needed later
                    qs = slice(q * PF, (q + 1) * PF)
                    nc.sync.dma_start(r_sb[:, qs], rq[:, c, sl][:, qs])
                    nc.sync.dma_start(w_sb[:, qs], wq[:, c, sl][:, qs])
                for q in range(pieces):
                    qs = slice(q * PF, (q + 1) * PF)
                    sgn = wtp.tile([128, PF], F32, tag=f"sgn{k}")
                    nc.scalar.activation(sgn[:], w_sb[:, qs], ACT.Sign)
                    s1 = wtp.tile([128, PF], F32, tag=f"s1{k}")
                    nc.vector.tensor_scalar(s1[:], r_sb[:, qs], TAU, C * TAU * ITAU,
                                            op0=ALU.is_lt, op1=ALU.mult)
                    s2 = wtp.tile([128, PF], F32, tag=f"s2{k}")
                    nc.vector.tensor_scalar(s2[:], r_sb[:, qs], TAU, C,
                                            op0=ALU.is_gt, op1=ALU.mult)
                    s3 = wtp.tile([128, PF], F32, tag=f"s3{k}")
                    nc.vector.tensor_add(s3[:], s1[:], s2[:])
                    s4 = wtp.tile([128, PF], F32, tag=f"s4{k}")
                    nc.vector.tensor_mul(s4[:], s3[:], sgn[:])
                    nc.vector.scalar_tensor_tensor(wp[:, qs], w_sb[:, qs], ITAU,
                                                   s4[:], op0=ALU.mult, op1=ALU.add)
                return wp

            def transposes(c, k, wp):
                # wp[p=co, i*9 + tap] -> lhsT[ci, co] per tap
                for t in range(9):
                    pt = tps.tile([128, 128], F32, tag="pt")
                    src = wp[:, t::9][:, :128]
                    nc.tensor.transpose(pt[:], src, ident[:])
                    nc.scalar.activation(
                        lhsT[:, (c * NCH + k) * 9 + t, :], pt[:], ACT.Copy)

            def prologue(c):
                # both chains emitted before any transposes: Sign(k1) on ACT
                # must outrank k0's transpose->lhsT copies or the k1 chain
                # (and with it the first full conv group) is delayed
                if c == 0:
                    # head-critical: k0 weights first, then x img0 quarters,
                    # then k1 weights. Early conv groups run their k0 taps
                    # while the k1 chain+transposes catch up.
                    wp0 = transform_chain(c, 0, pieces=2)
                    load_x(0, pieces=4)
                    wp1 = transform_chain(c, 1, pieces=2)
                else:
                    wp0 = transform_chain(c, 0)
                    wp1 = transform_chain(c, 1)
                transposes(c, 0, wp0)
                transposes(c, 1, wp1)

            def conv(c, img):
                for blk in range(NBLK):
                    h0 = blk * BLK
                    psum = cps.tile([128, TILE_N], F32, tag="psum")
                    i = 0
                    for k in range(NCH):
                        for dy, dx in TAP_ORDER:
                            r0 = max(h0, -dy)
                            r1 = min(h0 + BLK, H - dy)
                            nc.tensor.matmul(
                                psum[:, (r0 - h0) * W:(r1 - h0) * W],
                                lhsT[:, (c * NCH + k) * 9 + (dy + 1) * 3 + dx, :],
                                xv[img, k][:, r0 + dy:r1 + dy, dx:dx + W],
                                start=(i == 0), stop=(i == 2 * 9 - 1),
                            )
                            i += 1
                    ot = opool.tile([128, TILE_N], F32, tag="ot")
                    nc.vector.tensor_scalar_add(ot[:], psum[:],
                                                bias_sb[:, c:c + 1])
                    nc.sync.dma_start(
                        y_d[img, c * 128:(c + 1) * 128, h0:h0 + BLK], ot[:])

            # emission order ~ scheduling priority (DMA queue order and
            # engine pick order). w/r DMAs first so the transform chain
            # starts at ~3us; all c0 convs outrank the c1 prologue so DVE
            # drains (psum-bank frees) never starve behind transform ops.
            prologue(0)
            conv(0, 0)
            load_x(1)
            conv(0, 1)
            load_x(2)
            prologue(1)
            conv(0, 2)
            load_x(3)
            conv(0, 3)
            for img in range(B_SHARD):
                conv(1, img)

        if repeat == 1:
            body()
        else:
            tc.For_i_unrolled(0, repeat, 1, body, max_unroll=1)

    nc.compile()
    return nc


_CACHE = {}


def _get(repeat=1):
    if repeat not in _CACHE:
        _CACHE[repeat] = _build(repeat)
    return _CACHE[repeat]


def kernel(x, weight, bias, r_matrix):
    x = np.ascontiguousarray(np.asarray(x, dtype=np.float32))
    weight = np.ascontiguousarray(np.asarray(weight, dtype=np.float32))
    bias = np.ascontiguousarray(np.asarray(bias, dtype=np.float32))
    r_matrix = np.ascontiguousarray(np.asarray(r_matrix, dtype=np.float32))

    xp = np.pad(x, ((0, 0), (0, 0), (0, 0), (1, 1)))  # width 56 -> 58
    nc = _get(1)
    in_maps = [
        {"xp": xp[c * B_SHARD:(c + 1) * B_SHARD], "w": weight, "b": bias,
         "r": r_matrix}
        for c in range(N_CORES)
    ]
    res = run_bass_kernel_spmd(nc, in_maps, core_ids=list(range(N_CORES)))
    return np.concatenate([r["y"] for r in res.results], axis=0)

